# revision 1
# baseline (speedup 1.0000x reference)
"""Trainium2 Bass kernel for nms_detection (GaussianBlur5x5 -> MaxPool3x3 -> peak NMS + threshold).

Contract: kernel(hands_batch) takes the FULL [256, 2, 224, 398] f32 input and
returns the FULL [256, 2, 224, 398] f32 peaks map. Internally data-parallel
over 8 NeuronCores: 512 planes -> 64 planes/core.

Per-core algorithm (plane = one [224, 398] image channel):
  - Rows live on SBUF partitions; H=224 splits into two overlapping chunks of
    113 blur rows (+1 duplicated edge row -> M=114); 4 planes per supertile.
  - Blur: 3 accumulating fp32 matmuls per plane-chunk on the PE via gaussian
    symmetry gh=[a,b,c,b,a]: blur = (c*Gv)@x0 + (b*Gv)@s1 + (a*Gv)@s2 with
    s1=x[-1]+x[+1], s2=x[-2]+x[+2]. fp32 is mandatory: f32r/bf16/f16 matmuls
    measure ~11-bit mantissa on HW, flipping ~10k near-tie NMS compares (the
    2e-2 rel-err budget only allows ~500).
  - The s1/s2 pre-adds run on the Pool engine (gpsimd). Real-HW ISA limits
    Pool to add/mult/memset - max/is_ge/stt/copy ucodes are rejected by
    codegen (or silently return garbage) - so the ENTIRE max/compare chain
    must live on the DVE, Pool absorbs the adds + most of the final mult,
    and ACT does PSUM->SBUF blur copies + reflect edge-column scale-copies.
  - 3x3 maxpool: vertical max via two DMA-materialized partition-shifted
    copies (compute engines cannot read operands at mismatched partition
    offsets - BIR verifier rejects it); the shift-by-2 lands directly in the
    m2 tile whose cells are only overwritten later by the same engine.
    Horizontal max via free-dim shifted tensor_tensor + a fused
    scalar_tensor_tensor that folds in the detection threshold.
  - peaks_map = blur * [blur >= max(maxes, nextafter(THR))] (exact fp32
    compare); the masked VALUES are stored as bf16 and upcast on the host
    (0.4% value rounding, ~1.9e-3 rel-l2, well inside budget) halving
    output-store DMA traffic.
Schedule: 3-stage software pipeline (emission order = tile-scheduler
priority): loads+s1/s2 run PRE=2 steps ahead, matmuls+copies+shifts one step
ahead of the DVE chain, so no engine queue ever waits on a future load (an
op stuck in an in-order queue blocks everything behind it). Work is emitted
in 2-plane slices so independent chains interleave. Reflect edge columns are
folded into s1/s2 edge fixups emitted in the mm phase. DMA is batched
one-instruction-per-group via transposed DRAM access patterns.
TimelineSim: 340.3us e2e (DVE ~310 busy - the wall, only engine that can do
max/is_ge - Pool ~290, PE 255, DMA 226, ACT ~70) vs 368.1us baseline;
verified on 8-core HW: rel err 6.03e-3.
"""

import numpy as np

B, C, H, W = 256, 2, 224, 398
N_CORES = 8
PLANES = B * C                    # 512
P_CORE = PLANES // N_CORES        # 64 planes per core
GRP = 4                           # planes per supertile
KS = 5
SIGMA = 2.0
THR = 0.3

# chunk geometry: (raw_row0, out_row0)
#  chunk 0: blur rows 0..112   (ext: [b0, b0..b112]),  raw rows 0..114
#  chunk 1: blur rows 111..223 (ext: [b111..b223, b223]), raw rows 109..223
CHUNKS = [(0, 0), (109, 112)]
KDIM = 115                        # raw input rows per chunk
MDIM = 114                        # ext blur rows per chunk (113 + 1 dup)
OUTR = 112                        # output rows per chunk
WPAD = W + 4                      # reflect-padded width
FSPLIT = 157                       # final-mult column split (DVE left / Pool right)

_nc_cache = {}


def _gauss():
    x = np.arange(KS, dtype=np.float32) - np.float32((KS - 1) / 2.0)
    g = np.exp(np.float32(-0.5) * (x / np.float32(SIGMA)) ** 2).astype(np.float32)
    g = (g / g.sum()).astype(np.float32)
    return g


def _gmats():
    """lhsT matrices [2 chunks, 5 shifts, K=115, M=114] fp32, then packed
    to [115, 2*5*114] (partition dim = K first)."""
    g = _gauss()

    def refl(r):
        if r < 0:
            return -r
        if r >= H:
            return 2 * H - 2 - r
        return r

    out = np.zeros((2, KS, KDIM, MDIM), np.float32)
    for c, (raw0, _) in enumerate(CHUNKS):
        for m in range(MDIM):
            if c == 0:
                br = max(m - 1, 0)            # ext[0] duplicates blur row 0
            else:
                br = 111 + min(m, MDIM - 2)   # ext[113] duplicates blur row 223
            for i in range(KS):
                k = refl(br + i - 2) - raw0
                assert 0 <= k < KDIM
                for j in range(KS):
                    out[c, j, k, m] += g[i] * g[j]
    return np.ascontiguousarray(out.transpose(2, 0, 1, 3).reshape(KDIM, 2 * KS * MDIM))


def _build():
    import concourse.bacc as bacc
    import concourse.tile as tile
    import concourse.mybir as mybir

    f32 = mybir.dt.float32
    bf16 = mybir.dt.bfloat16
    AOT = mybir.AluOpType
    ACT = mybir.ActivationFunctionType
    THRP = float(np.nextafter(np.float32(THR), np.float32(1.0)))

    nc = bacc.Bacc(trn_type="TRN2", target_bir_lowering=False, debug=False)
    x_t = nc.dram_tensor("x", [P_CORE, H, W], f32, kind="ExternalInput")
    g_t = nc.dram_tensor("g", [KDIM, 2 * KS * MDIM], f32, kind="ExternalInput")
    o_t = nc.dram_tensor("o", [P_CORE, H, W], bf16, kind="ExternalOutput")
    x_ap = x_t.ap()
    o_ap = o_t.ap()

    NGRP = P_CORE // GRP
    IT = [(grp, ci) for grp in range(NGRP) for ci in range(2)]
    PRE = 2  # software-pipeline depth: produce runs PRE steps ahead of consume

    with tile.TileContext(nc) as tc:
        with tc.tile_pool(name="const", bufs=1) as constp, \
             tc.tile_pool(name="xin", bufs=PRE + 1) as xinp, \
             tc.tile_pool(name="ssum", bufs=PRE + 1) as ssump, \
             tc.tile_pool(name="work", bufs=3) as workp, \
             tc.tile_pool(name="ps", bufs=2, space="PSUM") as psp:
            gt = constp.tile([KDIM, 2 * KS * MDIM], f32, tag="g")
            nc.gpsimd.dma_start(out=gt[:], in_=g_t.ap())
            state = {}
            mmstate = {}

            def produce(it):
                grp, c = it
                raw0, _ = CHUNKS[c]
                p0 = grp * GRP
                # ---- load input tile (one batched DMA per group) ----
                xt = xinp.tile([KDIM, GRP, W], f32, tag="x")
                nc.sync.dma_start(
                    out=xt[:, :, :],
                    in_=x_ap[p0 : p0 + GRP, raw0 : raw0 + KDIM, :].transpose(
                        [1, 0, 2]
                    ),
                )
                # ---- shifted-sum tiles (exact fp32): s1[c]=x[c-1]+x[c+1],
                # s2[c]=x[c-2]+x[c+2]; horizontal reflect folds into the edge
                # columns as 2*x[k] (ACT scale-copies) or interior pairs (DVE).
                it_idx = IT.index(it)
                seng = nc.vector if it_idx < 2 else nc.gpsimd
                s1 = ssump.tile([KDIM, GRP, W], f32, tag="s1", name=f"s1_{grp}_{c}")
                seng.tensor_tensor(
                    s1[:, :, 1 : W - 1], xt[:, :, 0 : W - 2], xt[:, :, 2:W], AOT.add
                )
                s2 = ssump.tile([KDIM, GRP, W], f32, tag="s2", name=f"s2_{grp}_{c}")
                seng.tensor_tensor(
                    s2[:, :, 2 : W - 2], xt[:, :, 0 : W - 4], xt[:, :, 4:W], AOT.add
                )
                state[it] = (xt, s1, s2)

            def consume_mm(it):
                grp, c = it
                raw0, out0 = CHUNKS[c]
                p0 = grp * GRP
                xt, s1, s2 = state[it]
                first = grp == 0
                last = grp == NGRP - 1
                # reflect edge columns of s1/s2, emitted here (not in
                # produce) so no engine queue ever waits on a future load
                nc.scalar.activation(s1[:, :, 0:1], xt[:, :, 1:2], ACT.Copy, scale=2.0)
                nc.scalar.activation(
                    s1[:, :, W - 1 : W], xt[:, :, W - 2 : W - 1], ACT.Copy, scale=2.0
                )
                nc.scalar.activation(s2[:, :, 0:1], xt[:, :, 2:3], ACT.Copy, scale=2.0)
                nc.scalar.activation(
                    s2[:, :, W - 1 : W], xt[:, :, W - 3 : W - 2], ACT.Copy, scale=2.0
                )
                nc.gpsimd.tensor_tensor(
                    s2[:, :, 1:2], xt[:, :, 1:2], xt[:, :, 3:4], AOT.add
                )
                nc.gpsimd.tensor_tensor(
                    s2[:, :, W - 2 : W - 1], xt[:, :, W - 4 : W - 3],
                    xt[:, :, W - 2 : W - 1], AOT.add,
                )

                # ---- full separable blur on PE: 3 accumulating matmuls ----
                pss = [
                    psp.tile([MDIM, 512], f32, tag=f"p{i}", name=f"ps_{grp}_{c}_{i}")
                    for i in range(GRP)
                ]
                # j=2 (center, no s1/s2 dependency) first for overlap.  For
                # the first/last supertiles, go plane-outer (pipeline fill /
                # drain).
                order = (
                    [(j, i) for i in range(GRP) for j in (2, 1, 0)]
                    if (first or last)
                    else [(j, i) for j in (2, 1, 0) for i in range(GRP)]
                )
                for j, i in order:
                    term = (2, 1, 0).index(j)
                    lhs = gt[:, (c * KS + j) * MDIM : (c * KS + j + 1) * MDIM]
                    if j == 2:
                        rhs = xt[:, i, :]
                    elif j == 1:
                        rhs = s1[:, i, :]
                    else:
                        rhs = s2[:, i, :]
                    nc.tensor.matmul(
                        out=pss[i][:, 0:W],
                        lhsT=lhs,
                        rhs=rhs,
                        start=(term == 0),
                        stop=(term == 2),
                    )

                # ---- PSUM -> SBUF (ACT), plus shifted copies via DMA ----
                blur = workp.tile([MDIM, GRP, 400], f32, tag="blur")
                for i in range(GRP):
                    nc.scalar.copy(blur[:, i, 0:W], pss[i][:, 0:W])
                pl_slices = [slice(0, 2), slice(2, 4)]
                # blurdn[r] = ext[r+1]  (also the partition-aligned "valid
                # blur" tile: rows 0..111 = output rows); blurdn2[r] = ext[r+2]
                blurdn = workp.tile([MDIM - 1, GRP, 400], f32, tag="blurdn")
                for sl in pl_slices:
                    nc.sync.dma_start(
                        out=blurdn[:, sl, 0:W], in_=blur[1:MDIM, sl, 0:W]
                    )
                mmstate[it] = (blur, blurdn, pl_slices)

            def consume_chain(it):
                grp, c = it
                raw0, out0 = CHUNKS[c]
                p0 = grp * GRP
                first = grp == 0
                last = grp == NGRP - 1
                blur, blurdn, pl_slices = mmstate.pop(it)
                state.pop(it)
                # ---- 3x3 max + NMS (real-HW ISA: only the DVE implements
                # max/is_ge/stt; the Pool engine only has add/mult/memset, so
                # the whole compare chain lives on DVE and Pool absorbs the
                # s1/s2 pre-adds plus most of the final mult) ----
                t1 = workp.tile([MDIM - 1, GRP, 400], f32, tag="t1")
                # shift-2 lands directly in m2 (its cols are overwritten only
                # after vm reads them, same-engine in-order)
                m2 = workp.tile([OUTR, GRP, 400], f32, tag="m2")
                for sl in pl_slices:
                    nc.sync.dma_start(
                        out=m2[:, sl, 0:W], in_=blur[2:MDIM, sl, 0:W]
                    )
                vm = workp.tile([OUTR, GRP, 400], f32, tag="vm")
                t2 = t1[0:OUTR]
                mask = workp.tile([OUTR, GRP, 400], f32, tag="mask")
                outv = workp.tile([OUTR, GRP, 400], bf16, tag="outv")
                FC = 0 if IT.index((grp, c)) >= len(IT) - 11 else FSPLIT
                for sl in pl_slices:
                    nc.vector.tensor_tensor(
                        t1[:, sl, 0:W],
                        blur[0 : MDIM - 1, sl, 0:W],
                        blurdn[:, sl, 0:W],
                        AOT.max,
                    )
                    nc.vector.tensor_tensor(
                        vm[:, sl, 0:W],
                        t1[0:OUTR, sl, 0:W],
                        m2[:, sl, 0:W],
                        AOT.max,
                    )
                    nc.vector.tensor_tensor(
                        t2[:, sl, 0 : W - 1],
                        vm[:, sl, 0 : W - 1],
                        vm[:, sl, 1:W],
                        AOT.max,
                    )
                    nc.vector.scalar_tensor_tensor(
                        out=m2[:, sl, 1 : W - 1],
                        in0=t2[:, sl, 0 : W - 2],
                        scalar=THRP,
                        in1=t2[:, sl, 1 : W - 1],
                        op0=AOT.max,
                        op1=AOT.max,
                    )
                nc.vector.scalar_tensor_tensor(
                    out=m2[:, :, 0:1],
                    in0=t2[:, :, 0:1],
                    scalar=THRP,
                    in1=t2[:, :, 0:1],
                    op0=AOT.max,
                    op1=AOT.max,
                )
                nc.vector.scalar_tensor_tensor(
                    out=m2[:, :, W - 1 : W],
                    in0=t2[:, :, W - 2 : W - 1],
                    scalar=THRP,
                    in1=t2[:, :, W - 2 : W - 1],
                    op0=AOT.max,
                    op1=AOT.max,
                )
                for sl in pl_slices:
                    nc.vector.tensor_tensor(
                        mask[:, sl, 0:W],
                        blurdn[0:OUTR, sl, 0:W],
                        m2[:, sl, 0:W],
                        AOT.is_ge,
                    )
                    # outv = blur * mask, bf16: left cols on DVE, right on Pool
                    if FC > 0:
                        nc.vector.tensor_tensor(
                            outv[:, sl, 0:FC],
                            blurdn[0:OUTR, sl, 0:FC],
                            mask[:, sl, 0:FC],
                            AOT.mult,
                        )
                    if FC < W:
                        nc.gpsimd.tensor_tensor(
                            outv[:, sl, FC:W],
                            blurdn[0:OUTR, sl, FC:W],
                            mask[:, sl, FC:W],
                            AOT.mult,
                        )
                nc.sync.dma_start(
                    out=o_ap[p0 : p0 + GRP, out0 : out0 + OUTR, :].transpose(
                        [1, 0, 2]
                    ),
                    in_=outv[:, :, 0:W],
                )

            for step in range(len(IT) + PRE + 1):
                if step < len(IT):
                    produce(IT[step])
                if 1 <= step < len(IT) + 1:
                    consume_mm(IT[step - 1])
                if step >= PRE + 1:
                    consume_chain(IT[step - PRE - 1])

    nc.compile()
    return nc


def _make_sharded():
    """Build the shard_map'd PJRT executable ONCE and cache it, so repeat
    kernel() calls skip jit re-tracing / recompilation (~6s/call)."""
    import jax
    from jax.sharding import Mesh, PartitionSpec, NamedSharding
    from jax.experimental.shard_map import shard_map
    import concourse.mybir as mybir
    from concourse import bass2jax
    from concourse.bass2jax import _bass_exec_p, install_neuronx_cc_hook

    nc = _nc_cache["nc"]
    install_neuronx_cc_hook()
    partition_name = nc.partition_id_tensor.name if nc.partition_id_tensor else None
    in_names, out_names, out_avals, zero_shapes = [], [], [], []
    for alloc in nc.m.functions[0].allocations:
        if not isinstance(alloc, mybir.MemoryLocationSet):
            continue
        name = alloc.memorylocations[0].name
        if alloc.kind == "ExternalInput":
            if name != partition_name:
                in_names.append(name)
        elif alloc.kind == "ExternalOutput":
            out_names.append(name)
            shape = tuple(alloc.tensor_shape)
            dtype = mybir.dt.np(alloc.dtype)
            out_avals.append(jax.core.ShapedArray(shape, dtype))
            zero_shapes.append((shape, dtype))
    n_params = len(in_names)
    n_outs = len(out_avals)
    all_in_names = list(in_names) + list(out_names)
    if partition_name is not None:
        all_in_names.append(partition_name)

    def _body(*args):
        operands = list(args)
        if partition_name is not None:
            operands.append(bass2jax.partition_id_tensor())
        return tuple(_bass_exec_p.bind(
            *operands,
            out_avals=tuple(out_avals),
            in_names=tuple(all_in_names),
            out_names=tuple(out_names),
            lowering_input_output_aliases=(),
            sim_require_finite=True,
            sim_require_nnan=True,
            nc=nc,
        ))

    devices = jax.devices()[:N_CORES]
    mesh = Mesh(np.asarray(devices), ("core",))
    sharded = jax.jit(
        shard_map(
            _body, mesh=mesh,
            in_specs=(PartitionSpec("core"),) * (n_params + n_outs),
            out_specs=(PartitionSpec("core"),) * len(out_names),
            check_rep=False,
        ),
        donate_argnums=tuple(range(n_params, n_params + n_outs)),
        keep_unused=True,
    )
    sh = NamedSharding(mesh, PartitionSpec("core"))
    return sharded, sh, in_names, out_names, zero_shapes


def kernel(hands_batch: np.ndarray) -> np.ndarray:
    import jax

    x = np.ascontiguousarray(np.asarray(hands_batch, dtype=np.float32))
    assert x.shape == (B, C, H, W)

    if "nc" not in _nc_cache:
        _nc_cache["nc"] = _build()
        _nc_cache["g"] = _gmats()
        _nc_cache["fn"] = _make_sharded()
    sharded, sh, in_names, out_names, zero_shapes = _nc_cache["fn"]
    gm = _nc_cache["g"]

    concat = {
        "x": x.reshape(N_CORES * P_CORE, H, W),
        "g": np.concatenate([gm] * N_CORES, axis=0),
    }
    args = [jax.device_put(concat[nm], sh) for nm in in_names]
    zeros = [
        jax.device_put(np.zeros((N_CORES * s[0], *s[1:]), d), sh)
        for (s, d) in zero_shapes
    ]
    outs = sharded(*args, *zeros)
    out = np.asarray(outs[out_names.index("o")]).astype(np.float32)
    return out.reshape(B, C, H, W)


if __name__ == "__main__":
    rng = np.random.default_rng(0)
    x = rng.random((B, C, H, W), dtype=np.float32)
    y = kernel(x)
    print("kernel ran, out shape", y.shape, "nonzero frac", (y != 0).mean())

